# revision 22
# baseline (speedup 1.0000x reference)
"""Multi-head attention (B=4, S=2048, D=1024, H=16, causal) on 8 TRN2 NeuronCores.

Sharding: core c -> (batch b = c//2, head-group hg = c%2 of 8 heads).

v2 design (vs v1 baseline at ~647us):
- bf16 matmul operands everywhere (1.0 PE cycles/row vs 1.5 for f32r;
  halved LDWEIGHTS, SBUF and DMA). Accumulation stays fp32 in PSUM.
- xT built by DMA xbar transposes straight from DRAM (no PE/DVE work).
- Causal suffix restriction: fully-masked column blocks of diagonal
  score tiles are never computed, exp'd, or consumed (no gpsimd zeroing).
- Softmax denominator via ones-column in the att@V stationary; its
  reciprocal via the fast custom-DVE op on the [1,512] row, broadcast by
  a tiny PE matmul (replaces the 3.3us/instr DVE RECIPROCAL).
- Single interleaved instruction stream: attention (scores -> exp ->
  att@V with a one-step software-pipeline lag) for pair p is pumped with
  the QKV projection stream of pair p+1 (and with the output-projection
  chunks during the last pair) so the tensor engine never idles and the
  HAM clock gate stays at 8/8 (2.4 GHz).
"""

import sys
from collections import deque

import numpy as np
import ml_dtypes

for _p in ("/opt/trn_rl_repo", "/root/.axon_site/_ro/trn_rl_repo"):
    if _p not in sys.path:
        sys.path.append(_p)

import concourse.bass as bass
import concourse.tile as tile
from concourse import mybir
from concourse.bass_utils import run_bass_kernel_spmd

F32 = mybir.dt.float32
F32R = mybir.dt.float32r
BF16 = mybir.dt.bfloat16
BF16NP = ml_dtypes.bfloat16

B, S, D, H, HD = 4, 2048, 1024, 16, 64
P = 128
NPAIR = 4  # head pairs per core (8 heads)
NS = S // 512  # 4 s-runs of 512
NST = S // P  # 16 s-tiles of 128
NDC = D // P  # 8 d-chunks
SCALE = float(HD**-0.5)

_WAIT_EXEMPT = {
    "InstEventSemaphore",
    "InstUnconditionalBranch",
    "InstCall",
    "InstRegisterMove",
}


def fix_extra_waits(nc):
    """TRN2 compute-instruction structs encode at most one semaphore wait.
    After Tile scheduling, move extra waits onto engine NOPs inserted just
    before the over-constrained instruction (same engine, final order)."""
    import copy

    templates = {}

    def make_nop(engine):
        if engine not in templates:
            nc.engines[engine].nop()
            tail = nc.m.functions[0].blocks[-1]
            insts = tail.instructions
            templates[engine] = insts.pop()
            tail.instructions = insts
        nop = copy.deepcopy(templates[engine])
        nop.name = nc.get_next_instruction_name()
        return nop

    n_fixed = 0
    for fn in nc.m.functions:
        for blk in fn.blocks:
            out = []
            for inst in blk.instructions:
                si = getattr(inst, "sync_info", None)
                if (
                    type(inst).__name__ not in _WAIT_EXEMPT
                    and si is not None
                    and si.on_wait
                    and len(si.on_wait) > 1
                ):
                    waits = list(si.on_wait)
                    for w in waits[:-1]:
                        nop = make_nop(inst.engine)
                        nop.sync_info = mybir.SyncInfo(on_wait=[w], on_update=[])
                        out.append(nop)
                    si.on_wait = [waits[-1]]
                    n_fixed += 1
                out.append(inst)
            blk.instructions = out
    return n_fixed


def build_nc(apply_wait_fix=True):
    nc = bass.Bass()
    x_d = nc.dram_tensor("x", [S, D], BF16, kind="ExternalInput")
    wq_d = nc.dram_tensor("wq", [NPAIR, P, NDC, P], BF16, kind="ExternalInput")
    wk_d = nc.dram_tensor("wk", [NPAIR, P, NDC, P], BF16, kind="ExternalInput")
    wv_d = nc.dram_tensor("wv", [NPAIR, P, NDC, P], BF16, kind="ExternalInput")
    wp_d = nc.dram_tensor("wp", [P, NPAIR, D], BF16, kind="ExternalInput")
    ident_d = nc.dram_tensor("ident", [P, P], BF16, kind="ExternalInput")
    nones_d = nc.dram_tensor("nones", [1, 64], F32, kind="ExternalInput")
    ones_d = nc.dram_tensor("ones", [P, 32], BF16, kind="ExternalInput")
    tri01_d = nc.dram_tensor("tri01", [P, P], BF16, kind="ExternalInput")
    y_d = nc.dram_tensor("y", [S, D], F32, kind="ExternalOutput")

    with tile.TileContext(nc) as tc:
        with (
            tc.tile_pool(name="consts", bufs=1) as consts,
            tc.tile_pool(name="wpool", bufs=2) as wpool,
            tc.tile_pool(name="qk", bufs=2) as qk,
            tc.tile_pool(name="vpp", bufs=2) as vpp,
            tc.tile_pool(name="vtp", bufs=2) as vtp,
            tc.tile_pool(name="pex", bufs=6) as pex,
            tc.tile_pool(name="misc", bufs=2) as misc,
            tc.tile_pool(name="psS", bufs=2, space="PSUM") as psS,
            tc.tile_pool(name="psO", bufs=1, space="PSUM") as psO,
            tc.tile_pool(name="psM", bufs=2, space="PSUM") as psM,
        ):
            ident = consts.tile([P, P], BF16, tag="ident")
            nc.sync.dma_start(ident, ident_d[:, :])
            tri01 = consts.tile([P, P], BF16, tag="tri01")
            nc.sync.dma_start(tri01, tri01_d[:, :])
            nones1 = consts.tile([1, 64], F32R, tag="nones1")
            nc.sync.dma_start(nones1, nones_d[:, :].bitcast(F32R))
            # x transposed: [d-part, d-chunk, t], bf16
            xT = consts.tile([P, NDC, S], BF16, tag="xT")
            # normalized attention output, transposed: [pair-hk part, pair, s]
            OcatT = consts.tile([P, NPAIR, S], BF16, tag="OcatT")
            wp_sb = consts.tile([P, NPAIR, D], BF16, tag="wp")

            def _dma_wp():
                nc.sync.dma_start(wp_sb, wp_d[:, :, :])

            units = deque()

            def pump(k):
                n = 0
                while units and n < k:
                    units.popleft()()
                    n += 1

            def qkv_units(p):
                """Emission units for pair p's QKV projections (+ the global
                xT DMA transposes when include_x). Returns (QT, KT, Vp, U)."""
                QT = qk.tile([P, S], BF16, tag="QT")
                KT = qk.tile([P, S], BF16, tag="KT")
                Vp = vpp.tile([P, NST, 130], BF16, tag="Vp")
                Vp_r = Vp.rearrange("p t (two ko) -> p t two ko", two=2)
                w_sb = {}
                U = []
                for nm, wd in (("q", wq_d), ("k", wk_d), ("v", wv_d)):
                    w_sb[nm] = wpool.tile([P, NDC, P], BF16, tag="w" + nm, name="w" + nm)

                    def _dma_w(w_t=w_sb[nm], wd=wd):
                        nc.sync.dma_start(w_t, wd[p])

                    U.append(_dma_w)

                def _memset_ones():
                    nc.gpsimd.memset(Vp_r[:, :, :, 64:65], 1.0)

                U.append(_memset_ones)

                sc_units = [[] for _ in range(NS)]
                for sc in range(NS):
                    s0 = sc * 512
                    for nm in ("q", "k", "v"):
                        cell = {}
                        for dc in range(NDC):

                            def _mm(nm=nm, sc=sc, dc=dc, cell=cell, s0=s0):
                                if dc == 0:
                                    cell["ps"] = psM.tile(
                                        [P, 512], F32, tag="mm512", name="mm512"
                                    )
                                nc.tensor.matmul(
                                    cell["ps"],
                                    w_sb[nm][:, dc],
                                    xT[:, dc, s0 : s0 + 512],
                                    start=(dc == 0),
                                    stop=(dc == NDC - 1),
                                )
                                if dc == NDC - 1:
                                    if nm == "q":
                                        nc.vector.tensor_copy(
                                            out=QT[:, s0 : s0 + 512],
                                            in_=cell["ps"],
                                        )
                                    elif nm == "k":
                                        nc.vector.tensor_copy(
                                            out=KT[:, s0 : s0 + 512],
                                            in_=cell["ps"],
                                        )
                                    else:
                                        cell["vt"] = vtp.tile(
                                            [P, 512], BF16, tag="VT", name="VT"
                                        )
                                        nc.vector.tensor_copy(
                                            out=cell["vt"], in_=cell["ps"]
                                        )

                            sc_units[sc].append(_mm)
                        if nm == "v":
                            for k in range(4):

                                def _vtr(sc=sc, k=k, cell=cell):
                                    ptv = psM.tile([P, P], BF16, tag="mm512", name="ptv")
                                    nc.tensor.transpose(
                                        ptv,
                                        cell["vt"][:, k * P : (k + 1) * P],
                                        ident,
                                    )
                                    nc.vector.tensor_copy(
                                        out=Vp_r[:, sc * 4 + k, :, 0:64],
                                        in_=ptv.rearrange(
                                            "p (two k) -> p two k", two=2
                                        ),
                                    )

                                sc_units[sc].append(_vtr)
                return QT, KT, Vp, U, sc_units

            def attention(p, QT, KT, Vp, post_sr=None):
                for sr in range(NS):
                    n_t = 4 * (sr + 1)
                    n_tg = n_t // 2
                    s0 = sr * 512
                    po = {
                        h: psO.tile([65, 512], F32, tag=f"po{h}", name=f"po{h}")
                        for h in (0, 1)
                    }
                    ets = {}

                    def attv(h, tg):
                        et = ets.pop((h, tg))
                        for i in (0, 1):
                            tt = 2 * tg + i
                            j = tt - 4 * sr
                            c0 = 0 if j < 0 else 128 * j
                            nc.tensor.matmul(
                                po[h][:, c0:512],
                                Vp[:, tt, 65 * h : 65 * h + 65],
                                et[:, i, c0:512],
                                start=(tt == 0),
                                stop=(tt == n_t - 1),
                            )

                    for tg in range(n_tg):
                        for h in (0, 1):
                            pss = psS.tile([P, 2, 512], F32, tag="s")
                            for i in (0, 1):
                                tt = 2 * tg + i
                                j = tt - 4 * sr
                                c0 = 0 if j < 0 else 128 * j
                                nc.tensor.matmul(
                                    pss[:, i, c0:512],
                                    KT[64 * h : 64 * h + 64, tt * P : (tt + 1) * P],
                                    QT[64 * h : 64 * h + 64, s0 + c0 : s0 + 512],
                                    start=True,
                                    stop=True,
                                )
                            et = pex.tile([P, 2, 512], BF16, tag="e")
                            if 2 * tg + 1 < 4 * sr:
                                nc.scalar.activation(
                                    out=et,
                                    in_=pss,
                                    func=mybir.ActivationFunctionType.Exp,
                                    scale=SCALE,
                                )
                            else:
                                for i in (0, 1):
                                    j = 2 * tg + i - 4 * sr
                                    c0 = 0 if j < 0 else 128 * j
                                    nc.scalar.activation(
                                        out=et[:, i, c0:512],
                                        in_=pss[:, i, c0:512],
                                        func=mybir.ActivationFunctionType.Exp,
                                        scale=SCALE,
                                    )
                            # causal mask: zero the upper triangle of the
                            # diagonal block, post-exp (bf16 SBUF fast path)
                            for i in (0, 1):
                                j = 2 * tg + i - 4 * sr
                                if j >= 0:
                                    nc.vector.tensor_tensor(
                                        et[:, i, 128 * j : 128 * (j + 1)],
                                        et[:, i, 128 * j : 128 * (j + 1)],
                                        tri01,
                                        mybir.AluOpType.mult,
                                    )
                            pump(3)
                            if tg > 1:
                                attv(h, tg - 2)
                            ets[(h, tg)] = et
                    for h in (0, 1):
                        attv(h, n_tg - 2)
                        pump(1)
                    for h in (0, 1):
                        attv(h, n_tg - 1)
                        pump(1)
                    # 1/den = exp(-ln(den)); Ln and Exp share an ACT table.
                    # Both heads share one broadcast PSUM tile and one Exp.
                    # Normalization is deferred into the unit stream so
                    # the PE never sits behind a bcast waiting on ACT's Ln.
                    def _norm1(h, po_h, cell):
                        # 1/den = exp(-ln(den)); Ln/Exp share an ACT table
                        cell["lnr"] = misc.tile([1, 512], F32R, tag="lnr", name="lnr")
                        nc.scalar.activation(
                            out=cell["lnr"],
                            in_=po_h[64:65, :],
                            func=mybir.ActivationFunctionType.Ln,
                        )

                    def _norm2(h, po_h, cell, p=p, s0=s0):
                        pb = psM.tile([P, 512], F32, tag="mm512")
                        nc.tensor.matmul(
                            pb[0:64, :],
                            nones1,
                            cell["lnr"],
                            start=True,
                            stop=True,
                        )
                        rb = misc.tile([64, 512], F32, tag="rb")
                        nc.scalar.activation(
                            out=rb,
                            in_=pb[0:64, :],
                            func=mybir.ActivationFunctionType.Exp,
                        )
                        nc.vector.tensor_tensor(
                            OcatT[64 * h : 64 * h + 64, p, s0 : s0 + 512],
                            po_h[0:64, :],
                            rb,
                            mybir.AluOpType.mult,
                        )

                    import functools
                    for h in (1, 0):
                        cell = {}
                        units.appendleft(
                            functools.partial(_norm2, h, po[h], cell)
                        )
                        units.appendleft(
                            functools.partial(_norm1, h, po[h], cell)
                        )
                    if post_sr is not None:
                        post_sr(sr)

            def p3_units(sr):
                """Output-projection units for the 4 s-tiles of s-run sr."""
                U = []
                for st in range(4 * sr, 4 * sr + 4):
                    cell = {}
                    for dc2 in (0, 1):

                        def _mm(st=st, dc2=dc2, cell=cell):
                            if dc2 == 0:
                                cell["yt"] = misc.tile([P, D], F32, tag="yt", name="yt")
                            ps = psM.tile([P, 512], F32, tag="mm512")
                            for pp in range(NPAIR):
                                nc.tensor.matmul(
                                    ps,
                                    OcatT[:, pp, st * P : (st + 1) * P],
                                    wp_sb[:, pp, dc2 * 512 : (dc2 + 1) * 512],
                                    start=(pp == 0),
                                    stop=(pp == NPAIR - 1),
                                )
                            nc.vector.tensor_copy(
                                out=cell["yt"][:, dc2 * 512 : (dc2 + 1) * 512],
                                in_=ps,
                            )
                            if dc2 == 1:
                                nc.gpsimd.dma_start(
                                    y_d[st * P : (st + 1) * P, :], cell["yt"]
                                )

                        U.append(_mm)
                return U

            # ---- startup (all input DMAs on the one sync HWDGE queue):
            # wq -> xT half 0 -> wk, wv -> xT half 1 -> Vp-ones, wp.
            # Pair-0 chains for sc0/sc1 run inline; sc2/sc3 are deferred
            # into the attention(0) unit stream.
            def _xtr(half):
                for dc in range(NDC):
                    nc.sync.dma_start(
                        xT[:, dc, half * 1024 : (half + 1) * 1024],
                        x_d[half * 1024 : (half + 1) * 1024, dc * P : (dc + 1) * P],
                        transpose=True,
                    )

            QT, KT, Vp, U0, sc0_units = qkv_units(0)
            U0[0]()
            _xtr(0)
            U0[1]()
            U0[2]()
            _xtr(1)
            U0[3]()
            for u in sc0_units[0] + sc0_units[1]:
                u()
            _dma_wp()

            # Each pair's sc2/sc3 QKV chains are deferred into its OWN
            # attention phase (first needed by s-run 2), so even the last
            # pair's attention has pumpable tensor work.
            tail_units = sc0_units[2] + sc0_units[3]
            cur = (QT, KT, Vp)
            for p in range(NPAIR):
                units.extend(tail_units)
                if p < NPAIR - 1:
                    nxt = qkv_units(p + 1)
                    units.extend(nxt[3])
                    units.extend(nxt[4][0] + nxt[4][1])
                    tail_units = nxt[4][2] + nxt[4][3]
                    post = None
                else:
                    nxt = None

                    def post(sr):
                        units.extend(p3_units(sr))

                attention(p, *cur, post_sr=post)
                while units:
                    pump(1)
                if nxt is not None:
                    cur = nxt[:3]

    if apply_wait_fix:
        fix_extra_waits(nc)
    return nc


_NC = None


def _get_nc():
    global _NC
    if _NC is None:
        _NC = build_nc()
    return _NC


def _prep_core_inputs(x, Wq, Wk, Wv, Wp, core):
    b, hg = core // 2, core % 2
    hsl = slice(hg * 8, hg * 8 + 8)

    def prep_w(W):
        # [8, D, HD] -> [pair, dp, dc, (hip k)]
        a = W[hsl].reshape(NPAIR, 2, NDC, P, HD)
        return np.ascontiguousarray(
            a.transpose(0, 3, 2, 1, 4).reshape(NPAIR, P, NDC, P)
        ).astype(BF16NP)

    wp = np.ascontiguousarray(
        Wp[hg * 512 : (hg + 1) * 512]
        .reshape(NPAIR, P, D)
        .transpose(1, 0, 2)
    ).astype(BF16NP)

    return {
        "x": np.ascontiguousarray(x[b]).astype(BF16NP),
        "wq": prep_w(Wq),
        "wk": prep_w(Wk),
        "wv": prep_w(Wv),
        "wp": wp,
        "ident": np.eye(P, dtype=np.float32).astype(BF16NP),
        "nones": np.full((1, 64), -1.0, dtype=np.float32),
        "ones": np.ones((P, 32), dtype=np.float32).astype(BF16NP),
        "tri01": np.where(
            np.arange(P)[None, :] >= np.arange(P)[:, None], 1.0, 0.0
        ).astype(BF16NP),
    }


def kernel(trace=False, **inputs):
    x = np.asarray(inputs["x"], dtype=np.float32)
    Wq = np.asarray(inputs["Wq"], dtype=np.float32)
    Wk = np.asarray(inputs["Wk"], dtype=np.float32)
    Wv = np.asarray(inputs["Wv"], dtype=np.float32)
    Wp = np.asarray(inputs["Wp"], dtype=np.float32)
    bp = np.asarray(inputs["bp"], dtype=np.float32)

    nc = _get_nc()
    in_maps = [_prep_core_inputs(x, Wq, Wk, Wv, Wp, c) for c in range(8)]
    res = run_bass_kernel_spmd(nc, in_maps, core_ids=list(range(8)), trace=trace)

    out = np.empty((B, S, D), dtype=np.float32)
    for b in range(B):
        out[b] = res.results[2 * b]["y"] + res.results[2 * b + 1]["y"] + bp
    if trace:
        return out, res
    return out


# revision 23
# speedup vs baseline: 1.0248x; 1.0248x over previous
"""Multi-head attention (B=4, S=2048, D=1024, H=16, causal) on 8 TRN2 NeuronCores.

Sharding: core c -> (batch b = c//2, head-group hg = c%2 of 8 heads).

v2 design (vs v1 baseline at ~647us):
- bf16 matmul operands everywhere (1.0 PE cycles/row vs 1.5 for f32r;
  halved LDWEIGHTS, SBUF and DMA). Accumulation stays fp32 in PSUM.
- xT built by DMA xbar transposes straight from DRAM (no PE/DVE work).
- Causal suffix restriction: fully-masked column blocks of diagonal
  score tiles are never computed, exp'd, or consumed (no gpsimd zeroing).
- Softmax denominator via ones-column in the att@V stationary; its
  reciprocal via the fast custom-DVE op on the [1,512] row, broadcast by
  a tiny PE matmul (replaces the 3.3us/instr DVE RECIPROCAL).
- Single interleaved instruction stream: attention (scores -> exp ->
  att@V with a one-step software-pipeline lag) for pair p is pumped with
  the QKV projection stream of pair p+1 (and with the output-projection
  chunks during the last pair) so the tensor engine never idles and the
  HAM clock gate stays at 8/8 (2.4 GHz).
"""

import sys
from collections import deque

import numpy as np
import ml_dtypes

for _p in ("/opt/trn_rl_repo", "/root/.axon_site/_ro/trn_rl_repo"):
    if _p not in sys.path:
        sys.path.append(_p)

import concourse.bass as bass
import concourse.tile as tile
from concourse import mybir
from concourse.bass_utils import run_bass_kernel_spmd

F32 = mybir.dt.float32
F32R = mybir.dt.float32r
BF16 = mybir.dt.bfloat16
BF16NP = ml_dtypes.bfloat16

B, S, D, H, HD = 4, 2048, 1024, 16, 64
P = 128
NPAIR = 4  # head pairs per core (8 heads)
NS = S // 512  # 4 s-runs of 512
NST = S // P  # 16 s-tiles of 128
NDC = D // P  # 8 d-chunks
SCALE = float(HD**-0.5)

_WAIT_EXEMPT = {
    "InstEventSemaphore",
    "InstUnconditionalBranch",
    "InstCall",
    "InstRegisterMove",
}


def fix_extra_waits(nc):
    """TRN2 compute-instruction structs encode at most one semaphore wait.
    After Tile scheduling, move extra waits onto engine NOPs inserted just
    before the over-constrained instruction (same engine, final order)."""
    import copy

    templates = {}

    def make_nop(engine):
        if engine not in templates:
            nc.engines[engine].nop()
            tail = nc.m.functions[0].blocks[-1]
            insts = tail.instructions
            templates[engine] = insts.pop()
            tail.instructions = insts
        nop = copy.deepcopy(templates[engine])
        nop.name = nc.get_next_instruction_name()
        return nop

    n_fixed = 0
    for fn in nc.m.functions:
        for blk in fn.blocks:
            out = []
            for inst in blk.instructions:
                si = getattr(inst, "sync_info", None)
                if (
                    type(inst).__name__ not in _WAIT_EXEMPT
                    and si is not None
                    and si.on_wait
                    and len(si.on_wait) > 1
                ):
                    waits = list(si.on_wait)
                    for w in waits[:-1]:
                        nop = make_nop(inst.engine)
                        nop.sync_info = mybir.SyncInfo(on_wait=[w], on_update=[])
                        out.append(nop)
                    si.on_wait = [waits[-1]]
                    n_fixed += 1
                out.append(inst)
            blk.instructions = out
    return n_fixed


def build_nc(apply_wait_fix=True):
    nc = bass.Bass()
    x_d = nc.dram_tensor("x", [S, D], BF16, kind="ExternalInput")
    wq_d = nc.dram_tensor("wq", [NPAIR, P, NDC, P], BF16, kind="ExternalInput")
    wk_d = nc.dram_tensor("wk", [NPAIR, P, NDC, P], BF16, kind="ExternalInput")
    wv_d = nc.dram_tensor("wv", [NPAIR, P, NDC, P], BF16, kind="ExternalInput")
    wp_d = nc.dram_tensor("wp", [P, NPAIR, D], BF16, kind="ExternalInput")
    ident_d = nc.dram_tensor("ident", [P, P], BF16, kind="ExternalInput")
    nones_d = nc.dram_tensor("nones", [1, 64], F32, kind="ExternalInput")
    ones_d = nc.dram_tensor("ones", [P, 32], BF16, kind="ExternalInput")
    tri01_d = nc.dram_tensor("tri01", [P, P], BF16, kind="ExternalInput")
    y_d = nc.dram_tensor("y", [S, D], F32, kind="ExternalOutput")

    with tile.TileContext(nc) as tc:
        with (
            tc.tile_pool(name="consts", bufs=1) as consts,
            tc.tile_pool(name="wpool", bufs=2) as wpool,
            tc.tile_pool(name="qk", bufs=2) as qk,
            tc.tile_pool(name="vpp", bufs=2) as vpp,
            tc.tile_pool(name="vtp", bufs=2) as vtp,
            tc.tile_pool(name="pex", bufs=6) as pex,
            tc.tile_pool(name="misc", bufs=2) as misc,
            tc.tile_pool(name="psS", bufs=2, space="PSUM") as psS,
            tc.tile_pool(name="psO", bufs=1, space="PSUM") as psO,
            tc.tile_pool(name="psM", bufs=2, space="PSUM") as psM,
        ):
            ident = consts.tile([P, P], BF16, tag="ident")
            nc.gpsimd.dma_start(ident, ident_d[:, :])
            tri01 = consts.tile([P, P], BF16, tag="tri01")
            nc.gpsimd.dma_start(tri01, tri01_d[:, :])
            nones1 = consts.tile([1, 64], F32R, tag="nones1")
            nc.gpsimd.dma_start(nones1, nones_d[:, :].bitcast(F32R))
            # x transposed: [d-part, d-chunk, t], bf16
            xT = consts.tile([P, NDC, S], BF16, tag="xT")
            # normalized attention output, transposed: [pair-hk part, pair, s]
            OcatT = consts.tile([P, NPAIR, S], BF16, tag="OcatT")
            wp_sb = consts.tile([P, NPAIR, D], BF16, tag="wp")

            def _dma_wp():
                nc.gpsimd.dma_start(wp_sb, wp_d[:, :, :])

            units = deque()

            def pump(k):
                n = 0
                while units and n < k:
                    units.popleft()()
                    n += 1

            def qkv_units(p):
                """Emission units for pair p's QKV projections (+ the global
                xT DMA transposes when include_x). Returns (QT, KT, Vp, U)."""
                QT = qk.tile([P, S], BF16, tag="QT")
                KT = qk.tile([P, S], BF16, tag="KT")
                Vp = vpp.tile([P, NST, 130], BF16, tag="Vp")
                Vp_r = Vp.rearrange("p t (two ko) -> p t two ko", two=2)
                w_sb = {}
                U = []
                for nm, wd in (("q", wq_d), ("k", wk_d), ("v", wv_d)):
                    w_sb[nm] = wpool.tile([P, NDC, P], BF16, tag="w" + nm, name="w" + nm)

                    def _dma_w(w_t=w_sb[nm], wd=wd):
                        nc.gpsimd.dma_start(w_t, wd[p])

                    U.append(_dma_w)

                def _memset_ones():
                    nc.gpsimd.memset(Vp_r[:, :, :, 64:65], 1.0)

                U.append(_memset_ones)

                sc_units = [[] for _ in range(NS)]
                for sc in range(NS):
                    s0 = sc * 512
                    for nm in ("q", "k", "v"):
                        cell = {}
                        for dc in range(NDC):

                            def _mm(nm=nm, sc=sc, dc=dc, cell=cell, s0=s0):
                                if dc == 0:
                                    cell["ps"] = psM.tile(
                                        [P, 512], F32, tag="mm512", name="mm512"
                                    )
                                nc.tensor.matmul(
                                    cell["ps"],
                                    w_sb[nm][:, dc],
                                    xT[:, dc, s0 : s0 + 512],
                                    start=(dc == 0),
                                    stop=(dc == NDC - 1),
                                )
                                if dc == NDC - 1:
                                    if nm == "q":
                                        nc.vector.tensor_copy(
                                            out=QT[:, s0 : s0 + 512],
                                            in_=cell["ps"],
                                        )
                                    elif nm == "k":
                                        nc.vector.tensor_copy(
                                            out=KT[:, s0 : s0 + 512],
                                            in_=cell["ps"],
                                        )
                                    else:
                                        cell["vt"] = vtp.tile(
                                            [P, 512], BF16, tag="VT", name="VT"
                                        )
                                        nc.vector.tensor_copy(
                                            out=cell["vt"], in_=cell["ps"]
                                        )

                            sc_units[sc].append(_mm)
                        if nm == "v":
                            for k in range(4):

                                def _vtr(sc=sc, k=k, cell=cell):
                                    ptv = psM.tile([P, P], BF16, tag="mm512", name="ptv")
                                    nc.tensor.transpose(
                                        ptv,
                                        cell["vt"][:, k * P : (k + 1) * P],
                                        ident,
                                    )
                                    nc.vector.tensor_copy(
                                        out=Vp_r[:, sc * 4 + k, :, 0:64],
                                        in_=ptv.rearrange(
                                            "p (two k) -> p two k", two=2
                                        ),
                                    )

                                sc_units[sc].append(_vtr)
                return QT, KT, Vp, U, sc_units

            def attention(p, QT, KT, Vp, post_sr=None):
                for sr in range(NS):
                    n_t = 4 * (sr + 1)
                    n_tg = n_t // 2
                    s0 = sr * 512
                    po = {
                        h: psO.tile([65, 512], F32, tag=f"po{h}", name=f"po{h}")
                        for h in (0, 1)
                    }
                    ets = {}

                    def attv(h, tg):
                        et = ets.pop((h, tg))
                        for i in (0, 1):
                            tt = 2 * tg + i
                            j = tt - 4 * sr
                            c0 = 0 if j < 0 else 128 * j
                            nc.tensor.matmul(
                                po[h][:, c0:512],
                                Vp[:, tt, 65 * h : 65 * h + 65],
                                et[:, i, c0:512],
                                start=(tt == 0),
                                stop=(tt == n_t - 1),
                            )

                    for tg in range(n_tg):
                        for h in (0, 1):
                            pss = psS.tile([P, 2, 512], F32, tag="s")
                            for i in (0, 1):
                                tt = 2 * tg + i
                                j = tt - 4 * sr
                                c0 = 0 if j < 0 else 128 * j
                                nc.tensor.matmul(
                                    pss[:, i, c0:512],
                                    KT[64 * h : 64 * h + 64, tt * P : (tt + 1) * P],
                                    QT[64 * h : 64 * h + 64, s0 + c0 : s0 + 512],
                                    start=True,
                                    stop=True,
                                )
                            et = pex.tile([P, 2, 512], BF16, tag="e")
                            if 2 * tg + 1 < 4 * sr:
                                nc.scalar.activation(
                                    out=et,
                                    in_=pss,
                                    func=mybir.ActivationFunctionType.Exp,
                                    scale=SCALE,
                                )
                            else:
                                for i in (0, 1):
                                    j = 2 * tg + i - 4 * sr
                                    c0 = 0 if j < 0 else 128 * j
                                    nc.scalar.activation(
                                        out=et[:, i, c0:512],
                                        in_=pss[:, i, c0:512],
                                        func=mybir.ActivationFunctionType.Exp,
                                        scale=SCALE,
                                    )
                            # causal mask: zero the upper triangle of the
                            # diagonal block, post-exp (bf16 SBUF fast path)
                            for i in (0, 1):
                                j = 2 * tg + i - 4 * sr
                                if j >= 0:
                                    nc.vector.tensor_tensor(
                                        et[:, i, 128 * j : 128 * (j + 1)],
                                        et[:, i, 128 * j : 128 * (j + 1)],
                                        tri01,
                                        mybir.AluOpType.mult,
                                    )
                            pump(3)
                            if tg > 1:
                                attv(h, tg - 2)
                            ets[(h, tg)] = et
                    for h in (0, 1):
                        attv(h, n_tg - 2)
                        pump(1)
                    for h in (0, 1):
                        attv(h, n_tg - 1)
                        pump(1)
                    # 1/den = exp(-ln(den)); Ln and Exp share an ACT table.
                    # Both heads share one broadcast PSUM tile and one Exp.
                    # Normalization is deferred into the unit stream so
                    # the PE never sits behind a bcast waiting on ACT's Ln.
                    def _norm1(h, po_h, cell):
                        # 1/den = exp(-ln(den)); Ln/Exp share an ACT table
                        cell["lnr"] = misc.tile([1, 512], F32R, tag="lnr", name="lnr")
                        nc.scalar.activation(
                            out=cell["lnr"],
                            in_=po_h[64:65, :],
                            func=mybir.ActivationFunctionType.Ln,
                        )

                    def _norm2(h, po_h, cell, p=p, s0=s0):
                        pb = psM.tile([P, 512], F32, tag="mm512")
                        nc.tensor.matmul(
                            pb[0:64, :],
                            nones1,
                            cell["lnr"],
                            start=True,
                            stop=True,
                        )
                        rb = misc.tile([64, 512], F32, tag="rb")
                        nc.scalar.activation(
                            out=rb,
                            in_=pb[0:64, :],
                            func=mybir.ActivationFunctionType.Exp,
                        )
                        nc.vector.tensor_tensor(
                            OcatT[64 * h : 64 * h + 64, p, s0 : s0 + 512],
                            po_h[0:64, :],
                            rb,
                            mybir.AluOpType.mult,
                        )

                    import functools
                    for h in (1, 0):
                        cell = {}
                        units.appendleft(
                            functools.partial(_norm2, h, po[h], cell)
                        )
                        units.appendleft(
                            functools.partial(_norm1, h, po[h], cell)
                        )
                    if post_sr is not None:
                        post_sr(sr)

            def p3_units(sr):
                """Output-projection units for the 4 s-tiles of s-run sr."""
                U = []
                for st in range(4 * sr, 4 * sr + 4):
                    cell = {}
                    for dc2 in (0, 1):

                        def _mm(st=st, dc2=dc2, cell=cell):
                            if dc2 == 0:
                                cell["yt"] = misc.tile([P, D], F32, tag="yt", name="yt")
                            ps = psM.tile([P, 512], F32, tag="mm512")
                            for pp in range(NPAIR):
                                nc.tensor.matmul(
                                    ps,
                                    OcatT[:, pp, st * P : (st + 1) * P],
                                    wp_sb[:, pp, dc2 * 512 : (dc2 + 1) * 512],
                                    start=(pp == 0),
                                    stop=(pp == NPAIR - 1),
                                )
                            nc.vector.tensor_copy(
                                out=cell["yt"][:, dc2 * 512 : (dc2 + 1) * 512],
                                in_=ps,
                            )
                            if dc2 == 1:
                                nc.gpsimd.dma_start(
                                    y_d[st * P : (st + 1) * P, :], cell["yt"]
                                )

                        U.append(_mm)
                return U

            # ---- startup (all input DMAs on the one sync HWDGE queue):
            # wq -> xT half 0 -> wk, wv -> xT half 1 -> Vp-ones, wp.
            # Pair-0 chains for sc0/sc1 run inline; sc2/sc3 are deferred
            # into the attention(0) unit stream.
            def _xtr(half):
                for dc in range(NDC):
                    nc.sync.dma_start(
                        xT[:, dc, half * 1024 : (half + 1) * 1024],
                        x_d[half * 1024 : (half + 1) * 1024, dc * P : (dc + 1) * P],
                        transpose=True,
                    )

            QT, KT, Vp, U0, sc0_units = qkv_units(0)
            U0[0]()
            _xtr(0)
            U0[1]()
            U0[2]()
            _xtr(1)
            U0[3]()
            for u in sc0_units[0] + sc0_units[1]:
                u()
            _dma_wp()
            units.extend(sc0_units[2] + sc0_units[3])

            cur = (QT, KT, Vp)
            for p in range(NPAIR):
                if p < NPAIR - 1:
                    nxt = qkv_units(p + 1)
                    units.extend(nxt[3])
                    for scu in nxt[4]:
                        units.extend(scu)
                    post = None
                else:
                    nxt = None

                    def post(sr):
                        units.extend(p3_units(sr))

                attention(p, *cur, post_sr=post)
                while units:
                    pump(1)
                if nxt is not None:
                    cur = nxt[:3]

    if apply_wait_fix:
        fix_extra_waits(nc)
    return nc


_NC = None


def _get_nc():
    global _NC
    if _NC is None:
        _NC = build_nc()
    return _NC


def _prep_core_inputs(x, Wq, Wk, Wv, Wp, core):
    b, hg = core // 2, core % 2
    hsl = slice(hg * 8, hg * 8 + 8)

    def prep_w(W):
        # [8, D, HD] -> [pair, dp, dc, (hip k)]
        a = W[hsl].reshape(NPAIR, 2, NDC, P, HD)
        return np.ascontiguousarray(
            a.transpose(0, 3, 2, 1, 4).reshape(NPAIR, P, NDC, P)
        ).astype(BF16NP)

    wp = np.ascontiguousarray(
        Wp[hg * 512 : (hg + 1) * 512]
        .reshape(NPAIR, P, D)
        .transpose(1, 0, 2)
    ).astype(BF16NP)

    return {
        "x": np.ascontiguousarray(x[b]).astype(BF16NP),
        "wq": prep_w(Wq),
        "wk": prep_w(Wk),
        "wv": prep_w(Wv),
        "wp": wp,
        "ident": np.eye(P, dtype=np.float32).astype(BF16NP),
        "nones": np.full((1, 64), -1.0, dtype=np.float32),
        "ones": np.ones((P, 32), dtype=np.float32).astype(BF16NP),
        "tri01": np.where(
            np.arange(P)[None, :] >= np.arange(P)[:, None], 1.0, 0.0
        ).astype(BF16NP),
    }


def kernel(trace=False, **inputs):
    x = np.asarray(inputs["x"], dtype=np.float32)
    Wq = np.asarray(inputs["Wq"], dtype=np.float32)
    Wk = np.asarray(inputs["Wk"], dtype=np.float32)
    Wv = np.asarray(inputs["Wv"], dtype=np.float32)
    Wp = np.asarray(inputs["Wp"], dtype=np.float32)
    bp = np.asarray(inputs["bp"], dtype=np.float32)

    nc = _get_nc()
    in_maps = [_prep_core_inputs(x, Wq, Wk, Wv, Wp, c) for c in range(8)]
    res = run_bass_kernel_spmd(nc, in_maps, core_ids=list(range(8)), trace=trace)

    out = np.empty((B, S, D), dtype=np.float32)
    for b in range(B):
        out[b] = res.results[2 * b]["y"] + res.results[2 * b + 1]["y"] + bp
    if trace:
        return out, res
    return out


# revision 26
# speedup vs baseline: 1.1355x; 1.1079x over previous
"""Multi-head attention (B=4, S=2048, D=1024, H=16, causal) on 8 TRN2 NeuronCores.

Sharding: core c -> (batch b = c//2, head-group hg = c%2 of 8 heads).

v2 design (vs v1 baseline at ~647us):
- bf16 matmul operands everywhere (1.0 PE cycles/row vs 1.5 for f32r;
  halved LDWEIGHTS, SBUF and DMA). Accumulation stays fp32 in PSUM.
- xT built by DMA xbar transposes straight from DRAM (no PE/DVE work).
- Causal suffix restriction: fully-masked column blocks of diagonal
  score tiles are never computed, exp'd, or consumed (no gpsimd zeroing).
- Softmax denominator via ones-column in the att@V stationary; its
  reciprocal via the fast custom-DVE op on the [1,512] row, broadcast by
  a tiny PE matmul (replaces the 3.3us/instr DVE RECIPROCAL).
- Single interleaved instruction stream: attention (scores -> exp ->
  att@V with a one-step software-pipeline lag) for pair p is pumped with
  the QKV projection stream of pair p+1 (and with the output-projection
  chunks during the last pair) so the tensor engine never idles and the
  HAM clock gate stays at 8/8 (2.4 GHz).
"""

import sys
from collections import deque

import numpy as np
import ml_dtypes

for _p in ("/opt/trn_rl_repo", "/root/.axon_site/_ro/trn_rl_repo"):
    if _p not in sys.path:
        sys.path.append(_p)

import concourse.bass as bass
import concourse.tile as tile
from concourse import mybir
from concourse.bass_utils import run_bass_kernel_spmd

F32 = mybir.dt.float32
F32R = mybir.dt.float32r
BF16 = mybir.dt.bfloat16
BF16NP = ml_dtypes.bfloat16

B, S, D, H, HD = 4, 2048, 1024, 16, 64
P = 128
NPAIR = 4  # head pairs per core (8 heads)
NS = S // 512  # 4 s-runs of 512
NST = S // P  # 16 s-tiles of 128
NDC = D // P  # 8 d-chunks
SCALE = float(HD**-0.5)

_WAIT_EXEMPT = {
    "InstEventSemaphore",
    "InstUnconditionalBranch",
    "InstCall",
    "InstRegisterMove",
}


def fix_extra_waits(nc):
    """TRN2 compute-instruction structs encode at most one semaphore wait.
    After Tile scheduling, move extra waits onto engine NOPs inserted just
    before the over-constrained instruction (same engine, final order)."""
    import copy

    templates = {}

    def make_nop(engine):
        if engine not in templates:
            nc.engines[engine].nop()
            tail = nc.m.functions[0].blocks[-1]
            insts = tail.instructions
            templates[engine] = insts.pop()
            tail.instructions = insts
        nop = copy.deepcopy(templates[engine])
        nop.name = nc.get_next_instruction_name()
        return nop

    n_fixed = 0
    for fn in nc.m.functions:
        for blk in fn.blocks:
            out = []
            for inst in blk.instructions:
                si = getattr(inst, "sync_info", None)
                if (
                    type(inst).__name__ not in _WAIT_EXEMPT
                    and si is not None
                    and si.on_wait
                    and len(si.on_wait) > 1
                ):
                    waits = list(si.on_wait)
                    for w in waits[:-1]:
                        nop = make_nop(inst.engine)
                        nop.sync_info = mybir.SyncInfo(on_wait=[w], on_update=[])
                        out.append(nop)
                    si.on_wait = [waits[-1]]
                    n_fixed += 1
                out.append(inst)
            blk.instructions = out
    return n_fixed


def build_nc(apply_wait_fix=True):
    nc = bass.Bass()
    x_d = nc.dram_tensor("x", [S, D], BF16, kind="ExternalInput")
    wq_d = nc.dram_tensor("wq", [NPAIR, P, NDC, P], BF16, kind="ExternalInput")
    wk_d = nc.dram_tensor("wk", [NPAIR, P, NDC, P], BF16, kind="ExternalInput")
    wv_d = nc.dram_tensor("wv", [NPAIR, P, NDC, P], BF16, kind="ExternalInput")
    wp_d = nc.dram_tensor("wp", [P, NPAIR, D], BF16, kind="ExternalInput")
    ident_d = nc.dram_tensor("ident", [P, P], BF16, kind="ExternalInput")
    nones_d = nc.dram_tensor("nones", [1, 64], F32, kind="ExternalInput")
    tri01_d = nc.dram_tensor("tri01", [P, P], BF16, kind="ExternalInput")
    y_d = nc.dram_tensor("y", [S, D], F32, kind="ExternalOutput")

    with tile.TileContext(nc) as tc:
        with (
            tc.tile_pool(name="consts", bufs=1) as consts,
            tc.tile_pool(name="wpool", bufs=2) as wpool,
            tc.tile_pool(name="qk", bufs=2) as qk,
            tc.tile_pool(name="vpp", bufs=2) as vpp,
            tc.tile_pool(name="vtp", bufs=2) as vtp,
            tc.tile_pool(name="pex", bufs=6) as pex,
            tc.tile_pool(name="misc", bufs=2) as misc,
            tc.tile_pool(name="psS", bufs=2, space="PSUM") as psS,
            tc.tile_pool(name="psO", bufs=1, space="PSUM") as psO,
            tc.tile_pool(name="psM", bufs=2, space="PSUM") as psM,
        ):
            ident = consts.tile([P, P], BF16, tag="ident")
            tri01 = consts.tile([P, P], BF16, tag="tri01")
            nones1 = consts.tile([1, 64], F32R, tag="nones1")

            def _dma_consts():
                nc.sync.dma_start(ident, ident_d[:, :])
                nc.sync.dma_start(tri01, tri01_d[:, :])
                nc.sync.dma_start(nones1, nones_d[:, :].bitcast(F32R))
            # x transposed: [d-part, d-chunk, t], bf16
            xT = consts.tile([P, NDC, S], BF16, tag="xT")
            # normalized attention output, transposed: [pair-hk part, pair, s]
            OcatT = consts.tile([P, NPAIR, S], BF16, tag="OcatT")
            wp_sb = consts.tile([P, NPAIR, D], BF16, tag="wp")

            def _dma_wp():
                nc.sync.dma_start(wp_sb, wp_d[:, :, :])

            units = deque()

            def pump(k):
                n = 0
                while units and n < k:
                    units.popleft()()
                    n += 1

            def qkv_units(p):
                """Emission units for pair p's QKV projections (+ the global
                xT DMA transposes when include_x). Returns (QT, KT, Vp, U)."""
                QT = qk.tile([P, S], BF16, tag="QT")
                KT = qk.tile([P, S], BF16, tag="KT")
                Vp = vpp.tile([P, NST, 130], BF16, tag="Vp")
                Vp_r = Vp.rearrange("p t (two ko) -> p t two ko", two=2)
                w_sb = {}
                U = []
                for nm, wd in (("q", wq_d), ("k", wk_d), ("v", wv_d)):
                    w_sb[nm] = wpool.tile([P, NDC, P], BF16, tag="w" + nm, name="w" + nm)

                    def _dma_w(w_t=w_sb[nm], wd=wd):
                        nc.sync.dma_start(w_t, wd[p])

                    U.append(_dma_w)

                def _memset_ones():
                    nc.gpsimd.memset(Vp_r[:, :, :, 64:65], 1.0)

                U.append(_memset_ones)

                sc_units = [[] for _ in range(NS)]
                for sc in range(NS):
                    s0 = sc * 512
                    for nm in ("q", "k", "v"):
                        cell = {}
                        for dc in range(NDC):

                            def _mm(nm=nm, sc=sc, dc=dc, cell=cell, s0=s0):
                                if dc == 0:
                                    cell["ps"] = psM.tile(
                                        [P, 512], F32, tag="mm512", name="mm512"
                                    )
                                nc.tensor.matmul(
                                    cell["ps"],
                                    w_sb[nm][:, dc],
                                    xT[:, dc, s0 : s0 + 512],
                                    start=(dc == 0),
                                    stop=(dc == NDC - 1),
                                )
                                if dc == NDC - 1:
                                    if nm == "q":
                                        nc.vector.tensor_copy(
                                            out=QT[:, s0 : s0 + 512],
                                            in_=cell["ps"],
                                        )
                                    elif nm == "k":
                                        nc.vector.tensor_copy(
                                            out=KT[:, s0 : s0 + 512],
                                            in_=cell["ps"],
                                        )
                                    else:
                                        cell["vt"] = vtp.tile(
                                            [P, 512], BF16, tag="VT", name="VT"
                                        )
                                        nc.vector.tensor_copy(
                                            out=cell["vt"], in_=cell["ps"]
                                        )

                            sc_units[sc].append(_mm)
                        if nm == "v":
                            for k in range(4):

                                def _vtr(sc=sc, k=k, cell=cell):
                                    ptv = psM.tile([P, P], BF16, tag="mm512", name="ptv")
                                    nc.tensor.transpose(
                                        ptv,
                                        cell["vt"][:, k * P : (k + 1) * P],
                                        ident,
                                    )
                                    nc.vector.tensor_copy(
                                        out=Vp_r[:, sc * 4 + k, :, 0:64],
                                        in_=ptv.rearrange(
                                            "p (two k) -> p two k", two=2
                                        ),
                                    )

                                sc_units[sc].append(_vtr)
                return QT, KT, Vp, U, sc_units

            def attention(p, QT, KT, Vp, post_sr=None):
                for sr in range(NS):
                    n_t = 4 * (sr + 1)
                    n_tg = n_t // 2
                    s0 = sr * 512
                    po = {
                        h: psO.tile([65, 512], F32, tag=f"po{h}", name=f"po{h}")
                        for h in (0, 1)
                    }
                    ets = {}

                    def attv(h, tg):
                        et = ets.pop((h, tg))
                        for i in (0, 1):
                            tt = 2 * tg + i
                            j = tt - 4 * sr
                            c0 = 0 if j < 0 else 128 * j
                            nc.tensor.matmul(
                                po[h][:, c0:512],
                                Vp[:, tt, 65 * h : 65 * h + 65],
                                et[:, i, c0:512],
                                start=(tt == 0),
                                stop=(tt == n_t - 1),
                            )

                    for tg in range(n_tg):
                        for h in (0, 1):
                            pss = psS.tile([P, 2, 512], F32, tag="s")
                            for i in (0, 1):
                                tt = 2 * tg + i
                                j = tt - 4 * sr
                                c0 = 0 if j < 0 else 128 * j
                                nc.tensor.matmul(
                                    pss[:, i, c0:512],
                                    KT[64 * h : 64 * h + 64, tt * P : (tt + 1) * P],
                                    QT[64 * h : 64 * h + 64, s0 + c0 : s0 + 512],
                                    start=True,
                                    stop=True,
                                )
                            et = pex.tile([P, 2, 512], BF16, tag="e")
                            if 2 * tg + 1 < 4 * sr:
                                nc.scalar.activation(
                                    out=et,
                                    in_=pss,
                                    func=mybir.ActivationFunctionType.Exp,
                                    scale=SCALE,
                                )
                            else:
                                for i in (0, 1):
                                    j = 2 * tg + i - 4 * sr
                                    c0 = 0 if j < 0 else 128 * j
                                    nc.scalar.activation(
                                        out=et[:, i, c0:512],
                                        in_=pss[:, i, c0:512],
                                        func=mybir.ActivationFunctionType.Exp,
                                        scale=SCALE,
                                    )
                            # causal mask: zero the upper triangle of the
                            # diagonal block, post-exp (bf16 SBUF fast path)
                            for i in (0, 1):
                                j = 2 * tg + i - 4 * sr
                                if j >= 0:
                                    nc.vector.tensor_tensor(
                                        et[:, i, 128 * j : 128 * (j + 1)],
                                        et[:, i, 128 * j : 128 * (j + 1)],
                                        tri01,
                                        mybir.AluOpType.mult,
                                    )
                            pump(3)
                            if tg > 1:
                                attv(h, tg - 2)
                            ets[(h, tg)] = et
                    for h in (0, 1):
                        attv(h, n_tg - 2)
                        pump(1)
                    for h in (0, 1):
                        attv(h, n_tg - 1)
                        pump(1)
                    # 1/den = exp(-ln(den)); Ln and Exp share an ACT table.
                    # Both heads share one broadcast PSUM tile and one Exp.
                    # Normalization is deferred into the unit stream so
                    # the PE never sits behind a bcast waiting on ACT's Ln.
                    def _norm1(h, po_h, cell):
                        # 1/den = exp(-ln(den)); Ln/Exp share an ACT table
                        cell["lnr"] = misc.tile([1, 512], F32R, tag="lnr", name="lnr")
                        nc.scalar.activation(
                            out=cell["lnr"],
                            in_=po_h[64:65, :],
                            func=mybir.ActivationFunctionType.Ln,
                        )

                    def _norm2(h, po_h, cell, p=p, s0=s0):
                        pb = psM.tile([P, 512], F32, tag="mm512")
                        nc.tensor.matmul(
                            pb[0:64, :],
                            nones1,
                            cell["lnr"],
                            start=True,
                            stop=True,
                        )
                        rb = misc.tile([64, 512], F32, tag="rb")
                        nc.scalar.activation(
                            out=rb,
                            in_=pb[0:64, :],
                            func=mybir.ActivationFunctionType.Exp,
                        )
                        nc.vector.tensor_tensor(
                            OcatT[64 * h : 64 * h + 64, p, s0 : s0 + 512],
                            po_h[0:64, :],
                            rb,
                            mybir.AluOpType.mult,
                        )

                    import functools
                    for h in (1, 0):
                        cell = {}
                        units.appendleft(
                            functools.partial(_norm2, h, po[h], cell)
                        )
                        units.appendleft(
                            functools.partial(_norm1, h, po[h], cell)
                        )
                    if post_sr is not None:
                        post_sr(sr)

            def p3_units(sr):
                """Output-projection units for the 4 s-tiles of s-run sr."""
                U = []
                for st in range(4 * sr, 4 * sr + 4):
                    cell = {}
                    for dc2 in (0, 1):

                        def _mm(st=st, dc2=dc2, cell=cell):
                            if dc2 == 0:
                                cell["yt"] = misc.tile([P, D], F32, tag="yt", name="yt")
                            ps = psM.tile([P, 512], F32, tag="mm512")
                            for pp in range(NPAIR):
                                nc.tensor.matmul(
                                    ps,
                                    OcatT[:, pp, st * P : (st + 1) * P],
                                    wp_sb[:, pp, dc2 * 512 : (dc2 + 1) * 512],
                                    start=(pp == 0),
                                    stop=(pp == NPAIR - 1),
                                )
                            nc.vector.tensor_copy(
                                out=cell["yt"][:, dc2 * 512 : (dc2 + 1) * 512],
                                in_=ps,
                            )
                            if dc2 == 1:
                                nc.gpsimd.dma_start(
                                    y_d[st * P : (st + 1) * P, :], cell["yt"]
                                )

                        U.append(_mm)
                return U

            # ---- startup (all input DMAs on the one sync HWDGE queue):
            # wq -> xT half 0 -> wk, wv -> xT half 1 -> Vp-ones, wp.
            # Pair-0 chains for sc0/sc1 run inline; sc2/sc3 are deferred
            # into the attention(0) unit stream.
            def _xtr(half):
                for dc in range(NDC):
                    nc.sync.dma_start(
                        xT[:, dc, half * 1024 : (half + 1) * 1024],
                        x_d[half * 1024 : (half + 1) * 1024, dc * P : (dc + 1) * P],
                        transpose=True,
                    )

            QT, KT, Vp, U0, sc0_units = qkv_units(0)
            U0[0]()
            _xtr(0)
            _dma_consts()
            U0[1]()
            U0[2]()
            _xtr(1)
            U0[3]()
            for u in sc0_units[0] + sc0_units[1]:
                u()
            _dma_wp()
            units.extend(sc0_units[2] + sc0_units[3])

            cur = (QT, KT, Vp)
            for p in range(NPAIR):
                if p < NPAIR - 1:
                    nxt = qkv_units(p + 1)
                    units.extend(nxt[3])
                    for scu in nxt[4]:
                        units.extend(scu)
                    post = None
                else:
                    nxt = None

                    def post(sr):
                        units.extend(p3_units(sr))

                attention(p, *cur, post_sr=post)
                while units:
                    pump(1)
                if nxt is not None:
                    cur = nxt[:3]

    if apply_wait_fix:
        fix_extra_waits(nc)
    return nc


_NC = None


def _get_nc():
    global _NC
    if _NC is None:
        _NC = build_nc()
    return _NC


def _prep_core_inputs(x, Wq, Wk, Wv, Wp, core):
    b, hg = core // 2, core % 2
    hsl = slice(hg * 8, hg * 8 + 8)

    def prep_w(W):
        # [8, D, HD] -> [pair, dp, dc, (hip k)]
        a = W[hsl].reshape(NPAIR, 2, NDC, P, HD)
        return np.ascontiguousarray(
            a.transpose(0, 3, 2, 1, 4).reshape(NPAIR, P, NDC, P)
        ).astype(BF16NP)

    wp = np.ascontiguousarray(
        Wp[hg * 512 : (hg + 1) * 512]
        .reshape(NPAIR, P, D)
        .transpose(1, 0, 2)
    ).astype(BF16NP)

    return {
        "x": np.ascontiguousarray(x[b]).astype(BF16NP),
        "wq": prep_w(Wq),
        "wk": prep_w(Wk),
        "wv": prep_w(Wv),
        "wp": wp,
        "ident": np.eye(P, dtype=np.float32).astype(BF16NP),
        "nones": np.full((1, 64), -1.0, dtype=np.float32),
        "tri01": np.where(
            np.arange(P)[None, :] >= np.arange(P)[:, None], 1.0, 0.0
        ).astype(BF16NP),
    }


def kernel(trace=False, **inputs):
    x = np.asarray(inputs["x"], dtype=np.float32)
    Wq = np.asarray(inputs["Wq"], dtype=np.float32)
    Wk = np.asarray(inputs["Wk"], dtype=np.float32)
    Wv = np.asarray(inputs["Wv"], dtype=np.float32)
    Wp = np.asarray(inputs["Wp"], dtype=np.float32)
    bp = np.asarray(inputs["bp"], dtype=np.float32)

    nc = _get_nc()
    in_maps = [_prep_core_inputs(x, Wq, Wk, Wv, Wp, c) for c in range(8)]
    res = run_bass_kernel_spmd(nc, in_maps, core_ids=list(range(8)), trace=trace)

    out = np.empty((B, S, D), dtype=np.float32)
    for b in range(B):
        out[b] = res.results[2 * b]["y"] + res.results[2 * b + 1]["y"] + bp
    if trace:
        return out, res
    return out
